# revision 34
# baseline (speedup 1.0000x reference)
"""CMAttention Trainium2 kernel (8-core SPMD).

Reference computation (per nn_CMAttention):
  q_x = (x @ Wq_x.T)  -> [b, 16, n, 64],  q_a likewise
  kv_x = x @ Wkv_x.T -> k_x, v_x [b, 1, n, 64] (single shared KV head), kv_a likewise
  l2norm + learned scales on q_x/q_a (per head) and k_x/k_a (shared)
  q = concat(q_x, q_a) [b,16,n,128]; k, v likewise [b,1,n,128]
  rotary(q, k) over the 128-dim concat axis; SDPA with softmax over keys.

Sharding: heads are split across the 8 cores (2 heads per core, both batches);
the shared KV projection is computed replicated on every core.

Device-side layout: everything is computed "transposed" (feature dim on
partitions, sequence on the free axis); the host passes x/a pre-transposed
and reassembles the output. Softmax runs on S^T tiles: the row-sum over keys
is a ones-matmul partition reduction accumulated in PSUM; no max subtraction
is needed because q/k rows are l2-normalized (|scores*scale| <= ~0.2 for
unit scales; fp32 exp is safe far beyond that).

Performance notes:
  - All matmul operands are bf16 (1 PE cycle/row vs fp32's 4); PSUM stays
    fp32. Inputs/weights/rotary tables are cast to bf16 on the host, which
    also halves the input DMA traffic.
  - The learned qk scales are folded into per-head rotary tables on the
    host (valid because l2norm's 1/||q|| is uniform across the head dim and
    commutes with the rotation), so the norm is a plain multiply by
    1/sqrt(sum q^2), computed with a single fused Abs_reciprocal_sqrt
    activation (Copy/Square/AbsRsqrt share one Act table; Exp is the only
    table switch, once, at attention start).
  - V is transposed with one bf16 XBAR DMA-transpose per half (3D output
    splits the key index into (jt, partition)); no PE/PSUM/DVE cost.
  - Both heads' rotary packs are single DMAs ([P, 2, n] head-minor tiles);
    Walrus requires identical start partitions for all DVE operands, so
    these half-swaps must go through SBUF->SBUF DMA.
  - Program order per batch: proj -> vtrans -> norm -> rotary, then both
    attentions. pj(4 banks) + nm(4) coexist in PSUM; attention then takes
    all 8 (512-wide score chunks, bufs=4, so exp never stalls the PE).
"""

import numpy as np
from contextlib import ExitStack

import concourse.bass as bass
from concourse import bacc
import concourse.mybir as mybir
import concourse.tile as tile

F32 = mybir.dt.float32
BF16 = mybir.dt.bfloat16
AF = mybir.ActivationFunctionType
ALU = mybir.AluOpType

P = 128
B, N, DIM = 2, 2048, 1024
HEADS, DH, ROT = 16, 64, 128
NCORES, HPC = 8, 2          # 2 heads per core
KT = DIM // P               # 8 contraction tiles
SM_SCALE = float(1.0 / np.sqrt(ROT))


def build_nc(n=N, nb=B):
    CH = min(512, n)        # matmul moving-operand chunk (1 PSUM bank fp32)
    NCH = n // CH
    SU = min(1024, n)       # attention superunit width (2 PSUM banks)
    NSU = n // SU
    SUC = SU // CH
    NJT = n // P            # key tiles

    nc = bacc.Bacc()
    dp = nc.declare_dram_parameter
    xT = dp("xT", [nb, DIM, n], BF16, isOutput=False)
    aT = dp("aT", [nb, DIM, n], BF16, isOutput=False)
    wqx = dp("wqx", [DIM, P], BF16, isOutput=False)
    wqa = dp("wqa", [DIM, P], BF16, isOutput=False)
    wkvx = dp("wkvx", [DIM, P], BF16, isOutput=False)   # cols [k_x | v_x]
    wkva = dp("wkva", [DIM, P], BF16, isOutput=False)   # cols [v_a | k_a] (host-permuted)
    cosq = dp("cosq", [P, HPC, n], BF16, isOutput=False)   # cos * q-scale, per head
    sinq = dp("sinq", [P, HPC, n], BF16, isOutput=False)   # signed sin * swapped q-scale
    cosk = dp("cosk", [P, n], BF16, isOutput=False)
    sink = dp("sink", [P, n], BF16, isOutput=False)
    out = dp("out", [nb, HPC, ROT, n], F32, isOutput=True)

    with ExitStack() as ctx:
        tc = ctx.enter_context(tile.TileContext(nc))
        consts = ctx.enter_context(tc.tile_pool(name="consts", bufs=1))
        sb = ctx.enter_context(tc.tile_pool(name="sb", bufs=1))

        ones = consts.tile([P, P], BF16)
        nc.vector.memset(ones, 1.0)
        eps_sb = consts.tile([P, 1], F32)
        nc.vector.memset(eps_sb, 1e-24)

        cosq_sb = consts.tile([P, HPC, n], BF16)
        nc.gpsimd.dma_start(out=cosq_sb, in_=cosq[:])
        sinq_sb = consts.tile([P, HPC, n], BF16)
        nc.gpsimd.dma_start(out=sinq_sb, in_=sinq[:])
        cosk_sb = consts.tile([P, n], BF16)
        nc.gpsimd.dma_start(out=cosk_sb, in_=cosk[:])
        sink_sb = consts.tile([P, n], BF16)
        nc.gpsimd.dma_start(out=sink_sb, in_=sink[:])

        w_sb = {}
        for name, hdl in (("wqx", wqx), ("wqa", wqa), ("wkvx", wkvx), ("wkva", wkva)):
            t = consts.tile([P, KT, P], BF16, name=f"w_{name}")
            nc.gpsimd.dma_start(out=t, in_=hdl[:].rearrange("(kt p) m -> p kt m", p=P))
            w_sb[name] = t

        # per-batch state
        QX, QA, KVX, KVA = {}, {}, {}, {}
        V = {}             # per b: [P, NJT, P] transposed values
        QH = {}            # per b: [P, HPC, n] packed+rotated q (both heads)
        KN = {}            # per b: packed+rotated k

        KTS = {}           # (b, src-name) -> list of input tiles

        def load(b):
            # issue all of batch b's input-tile DMAs; hoisted so the SP
            # HWDGE FIFO is never head-of-line blocked by later transfers
            # whose dependencies resolve late.
            for nmsrc, src in (("x", xT), ("a", aT)):
                kts = []
                for ki in range(KT):
                    t = sb.tile([P, n], BF16, tag="ktile", bufs=10)
                    nc.sync.dma_start(out=t, in_=src[b, ki * P:(ki + 1) * P, :])
                    kts.append(t)
                KTS[(b, nmsrc)] = kts

        def proj_pass(b, srcname, pj):
            kts = KTS[(b, srcname)]
            if srcname == "x":
                wq_t, wkv_t = w_sb["wqx"], w_sb["wkvx"]
                qdst = QX[b] = sb.tile([P, n], BF16, tag="proj", bufs=8,
                                       name=f"QX{b}")
                kvdst = KVX[b] = sb.tile([P, n], BF16, tag="proj", bufs=8,
                                         name=f"KVX{b}")
            else:
                wq_t, wkv_t = w_sb["wqa"], w_sb["wkva"]
                qdst = QA[b] = sb.tile([P, n], BF16, tag="proj", bufs=8,
                                       name=f"QA{b}")
                kvdst = KVA[b] = sb.tile([P, n], BF16, tag="proj", bufs=8,
                                         name=f"KVA{b}")
            for c in range(NCH):
                cs = slice(c * CH, (c + 1) * CH)
                psQ = pj.tile([P, CH], F32, tag="pq", bufs=2)
                psK = pj.tile([P, CH], F32, tag="pk", bufs=2)
                for ki, t in enumerate(kts):
                    nc.tensor.matmul(psQ, wq_t[:, ki, :], t[:, cs],
                                     start=(ki == 0), stop=(ki == KT - 1))
                    nc.tensor.matmul(psK, wkv_t[:, ki, :], t[:, cs],
                                     start=(ki == 0), stop=(ki == KT - 1))
                # PSUM->SBUF eviction on the Activation engine, which is
                # otherwise idle before attention; keeps DVE free.
                nc.scalar.activation(qdst[:, cs], psQ, AF.Copy,
                                     bias=0.0, scale=1.0)
                nc.scalar.activation(kvdst[:, cs], psK, AF.Copy,
                                     bias=0.0, scale=1.0)

        def vtrans(b):
            # V[key, d]: cols 0:64 = v_x (KVX rows 64:128), cols 64:128 = v_a
            # (KVA rows 0:64).  bf16 XBAR DMA transpose, one instruction per
            # half: out[p, jt, d] = in[d, jt*128 + p].
            va = sb.tile([P, NJT, P], BF16, tag="vsb", bufs=2, name=f"v{b}")
            nc.sync.dma_start_transpose(out=va[:, :, 0:DH], in_=KVX[b][DH:P, :])
            nc.sync.dma_start_transpose(out=va[:, :, DH:P], in_=KVA[b][0:DH, :])
            V[b] = va

        def norm_pass(b, srcname, nm):
            # multiply by 1/sqrt(sum q^2 + eps); the learned scales live in
            # the rotary tables.  Q tensors normalized in place, K halves
            # written into the packed KN tile.  Called per projection pass
            # so each stream's Act work starts as soon as its source tile
            # is evicted from PSUM.
            if (b, "kn") not in KTS:
                KTS[(b, "kn")] = KN[b] = sb.tile([P, n], BF16, tag="kn",
                                                 bufs=2, name=f"KN{b}")
            if srcname == "x":
                streams = ((QX[b], QX[b], (0, P)), (KVX[b], KN[b], (0, DH)))
            else:
                streams = ((QA[b], QA[b], (0, P)), (KVA[b], KN[b], (DH, P)))
            CHN = min(2 * CH, n)   # norm chunk: <=2 PSUM banks per psr buffer
            for src, dst, (r0, r1) in streams:
                q2 = sb.tile([P, n], BF16, tag="q2", bufs=2)
                rcp = sb.tile([P, n], BF16, tag="rcp", bufs=2)
                nc.vector.tensor_mul(q2[r0:r1, :], src[r0:r1, :], src[r0:r1, :])
                for c in range(n // CHN):
                    cs = slice(c * CHN, (c + 1) * CHN)
                    psr = nm.tile([P, CHN], F32, tag="r", bufs=2)
                    for h0 in range(r0, r1, DH):
                        h1 = h0 + DH
                        # matmul outputs must stay within one PSUM bank
                        for cc in range(CHN // CH):
                            el = slice(cc * CH, (cc + 1) * CH)
                            qs = slice(c * CHN + cc * CH, c * CHN + (cc + 1) * CH)
                            nc.tensor.matmul(psr[h0:h1, el], ones[h0:h1, 0:DH],
                                             q2[h0:h1, qs], start=True, stop=True)
                    # fused 1/sqrt on the Act engine (same table as Copy and
                    # Square -> no act-table reloads in the whole front)
                    nc.scalar.activation(rcp[r0:r1, cs], psr[r0:r1, :],
                                         AF.Abs_reciprocal_sqrt,
                                         bias=eps_sb[r0:r1, :], scale=1.0)
                nc.vector.tensor_mul(dst[r0:r1, :], src[r0:r1, :], rcp[r0:r1, :])

        def rotary(b):
            # rot(t)[0:64] = t[0:64]*cos64 - t[64:128]*sin64
            # rot(t)[64:128] = t[64:128]*cos64 + t[0:64]*sin64
            # The tables carry the sign and the learned scales. Walrus needs
            # identical start partitions for DVE operands, so the half-swapped
            # companions are built with SBUF->SBUF DMAs (one per source
            # covering both heads; dest is head-minor [P, HPC, n]).
            qh = sb.tile([P, HPC, n], BF16, tag="qh", bufs=2, name=f"qh{b}")
            qsw = sb.tile([P, HPC, n], BF16, tag="qsw", bufs=1, name=f"qsw{b}")
            # the SP HWDGE FIFO is drained of input tiles by the time these
            # are ready (loads are hoisted), so SP carries the packs.
            # (SBUF APs cannot move the partition axis into a free dim, so
            # this is one DMA per source half per head.)
            for h in range(HPC):
                hs = slice(h * DH, (h + 1) * DH)
                nc.sync.dma_start(out=qh[0:DH, h, :], in_=QX[b][hs, :])
                nc.sync.dma_start(out=qh[DH:P, h, :], in_=QA[b][hs, :])
                nc.sync.dma_start(out=qsw[0:DH, h, :], in_=QA[b][hs, :])
                nc.sync.dma_start(out=qsw[DH:P, h, :], in_=QX[b][hs, :])
            tc_ = sb.tile([P, HPC, n], BF16, tag="rt", bufs=1)
            nc.vector.tensor_mul(tc_, qh, cosq_sb)
            nc.vector.tensor_mul(qsw, qsw, sinq_sb)
            nc.vector.tensor_add(qh, tc_, qsw)
            QH[b] = qh

            ksw = sb.tile([P, n], BF16, tag="ksw", bufs=2, name=f"ksw{b}")
            nc.sync.dma_start(out=ksw[0:DH, :], in_=KN[b][DH:P, :])
            nc.sync.dma_start(out=ksw[DH:P, :], in_=KN[b][0:DH, :])
            tck = sb.tile([P, n], BF16, tag="rtk", bufs=1)
            nc.vector.tensor_mul(tck, KN[b], cosk_sb)
            nc.vector.tensor_mul(ksw, ksw, sink_sb)
            nc.vector.tensor_add(KN[b], tck, ksw)

        def attn(b, at):
            krot = KN[b]
            for h in range(HPC):
                qr = QH[b]
                for su in range(NSU):
                    ps_o = at.tile([P, SU], F32, tag="o", bufs=1)
                    # softmax denominator: accumulate sum_jt(es) with cheap
                    # bf16 DVE adds, then ONE ones-matmul partition-reduce on
                    # the pre-summed tile -- removes the per-tile rowsum
                    # matmul (1/3 of attention PE work).
                    acc = sb.tile([P, SU], BF16, tag="acc", bufs=2)
                    for jt in range(NJT):
                        js = slice(jt * P, (jt + 1) * P)
                        ps_s = at.tile([P, SU], F32, tag="s", bufs=3)
                        for cc in range(SUC):
                            el = slice(cc * CH, (cc + 1) * CH)
                            il = slice(su * SU + cc * CH, su * SU + (cc + 1) * CH)
                            nc.tensor.matmul(ps_s[:, el], krot[:, js],
                                             qr[:, h, il], start=True, stop=True)
                        es = sb.tile([P, SU], BF16, tag="es", bufs=3)
                        nc.scalar.activation(es, ps_s, AF.Exp, bias=0.0,
                                             scale=SM_SCALE)
                        if jt == 0:
                            nc.vector.tensor_copy(acc, es)
                        else:
                            nc.vector.tensor_add(acc, acc, es)
                        for cc in range(SUC):
                            el = slice(cc * CH, (cc + 1) * CH)
                            nc.tensor.matmul(ps_o[:, el], V[b][:, jt, :],
                                             es[:, el],
                                             start=(jt == 0), stop=(jt == NJT - 1))
                    psd = at.tile([P, SU], F32, tag="s", bufs=3)
                    for cc in range(SUC):
                        el = slice(cc * CH, (cc + 1) * CH)
                        nc.tensor.matmul(psd[:, el], ones, acc[:, el],
                                         start=True, stop=True)
                    rec = sb.tile([P, SU], F32, tag="rec", bufs=1)
                    nc.vector.reciprocal(rec, psd)
                    on = sb.tile([P, SU], F32, tag="on", bufs=2)
                    nc.vector.tensor_mul(on, ps_o, rec)
                    nc.sync.dma_start(out=out[b, h, :, su * SU:(su + 1) * SU],
                                      in_=on)

        # ---- program order: per-batch proj -> vtrans -> norm -> rotary
        # (rotary right after its norm keeps the DVE FIFO from head-of-line
        # blocking batch 0's rotary behind batch 1's norm), then attention
        # (needs all 8 PSUM banks -> pj/nm must close first).
        with tc.tile_pool(name="pj", bufs=1, space="PSUM") as pj, \
             tc.tile_pool(name="nm", bufs=1, space="PSUM") as nm:
            for b in range(nb):
                load(b)
            for b in range(nb):
                proj_pass(b, "x", pj)
                norm_pass(b, "x", nm)
                proj_pass(b, "a", pj)
                norm_pass(b, "a", nm)
                vtrans(b)
                rotary(b)
        with tc.tile_pool(name="at", bufs=1, space="PSUM") as at:
            for b in range(nb):
                attn(b, at)
    nc.finalize()
    return nc


# ---------------------------------------------------------------------------
# host side
# ---------------------------------------------------------------------------

_NC_CACHE = {}


def get_nc(n=N, nb=B):
    key = (n, nb)
    if key not in _NC_CACHE:
        _NC_CACHE[key] = build_nc(n, nb)
    return _NC_CACHE[key]


def rotary_tables(n):
    inv_freq = 1.0 / (10000.0 ** (np.arange(0, ROT, 2, dtype=np.float64) / ROT))
    freqs = np.outer(np.arange(n, dtype=np.float64), inv_freq)  # [n, 64]
    cos64 = np.cos(freqs).T.astype(np.float32)                  # [64, n]
    sin64 = np.sin(freqs).T.astype(np.float32)
    cos2 = np.concatenate([cos64, cos64], 0)                    # [128, n]
    sin2 = np.concatenate([-sin64, sin64], 0)                   # signed
    return cos2, sin2


def _bf16(a):
    import ml_dtypes
    return np.ascontiguousarray(np.asarray(a, dtype=np.float32)).astype(
        ml_dtypes.bfloat16)


def prep_in_maps(inputs, n=N, nb=B, ncores=NCORES):
    g = {k: np.ascontiguousarray(np.asarray(v, dtype=np.float32))
         for k, v in inputs.items()}
    xT = _bf16(g["x"].transpose(0, 2, 1))
    aT = _bf16(g["a"].transpose(0, 2, 1))
    wkvx = _bf16(g["Wkv_x"].T)                                     # cols [kx|vx]
    wkva = _bf16(
        np.concatenate([g["Wkv_a"][DH:2 * DH], g["Wkv_a"][0:DH]], 0).T)  # cols [va|ka]
    cos2, sin2 = rotary_tables(n)

    # fold the learned qk scales into the rotary tables:
    #   rot(qn * sc)[d] = cos2[d]*sc[d]*qn[d] + sin2[d]*sc[sigma(d)]*qn[sigma(d)]
    # where sigma swaps the two 64-halves; qn is the unscaled l2-normalized q.
    sck = np.concatenate([g["kx_scale"][0, 0], g["ka_scale"][0, 0]])       # [128]
    swk = np.concatenate([g["ka_scale"][0, 0], g["kx_scale"][0, 0]])
    cosk = cos2 * sck[:, None]
    sink = sin2 * swk[:, None]

    shared = dict(xT=xT, aT=aT, wkvx=wkvx, wkva=wkva,
                  cosk=_bf16(cosk), sink=_bf16(sink))
    in_maps = []
    for c in range(ncores):
        h0 = c * HPC
        m = dict(shared)
        m["wqx"] = _bf16(g["Wq_x"][h0 * DH:(h0 + HPC) * DH].T)
        m["wqa"] = _bf16(g["Wq_a"][h0 * DH:(h0 + HPC) * DH].T)
        cq = np.empty((P, HPC, n), np.float32)
        sq = np.empty((P, HPC, n), np.float32)
        for h in range(HPC):
            scq = np.concatenate([g["qx_scale"][h0 + h, 0], g["qa_scale"][h0 + h, 0]])
            swq = np.concatenate([g["qa_scale"][h0 + h, 0], g["qx_scale"][h0 + h, 0]])
            cq[:, h, :] = cos2 * scq[:, None]
            sq[:, h, :] = sin2 * swq[:, None]
        m["cosq"] = _bf16(cq)
        m["sinq"] = _bf16(sq)
        in_maps.append(m)
    return in_maps


def gather_out(results, n=N, nb=B, ncores=NCORES):
    full = np.empty((nb, n, HEADS * ROT), np.float32)
    for c in range(ncores):
        o = np.asarray(results[c]["out"])          # [nb, HPC, ROT, n]
        for h in range(HPC):
            gh = c * HPC + h
            full[:, :, gh * ROT:(gh + 1) * ROT] = o[:, h].transpose(0, 2, 1)
    return full


def kernel(**inputs):
    from concourse.bass_utils import run_bass_kernel_spmd
    nc = get_nc(N, B)
    in_maps = prep_in_maps(inputs, N, B, NCORES)
    res = run_bass_kernel_spmd(nc, in_maps, list(range(NCORES)))
    return gather_out(res.results, N, B, NCORES)


if __name__ == "__main__":
    nc = build_nc(256, 1)
    print("build ok")


# revision 37
# speedup vs baseline: 28.9829x; 28.9829x over previous
"""CMAttention Trainium2 kernel (8-core SPMD).

Reference computation (per nn_CMAttention):
  q_x = (x @ Wq_x.T)  -> [b, 16, n, 64],  q_a likewise
  kv_x = x @ Wkv_x.T -> k_x, v_x [b, 1, n, 64] (single shared KV head), kv_a likewise
  l2norm + learned scales on q_x/q_a (per head) and k_x/k_a (shared)
  q = concat(q_x, q_a) [b,16,n,128]; k, v likewise [b,1,n,128]
  rotary(q, k) over the 128-dim concat axis; SDPA with softmax over keys.

Sharding: heads are split across the 8 cores (2 heads per core, both batches);
the shared KV projection is computed replicated on every core.

Device-side layout: everything is computed "transposed" (feature dim on
partitions, sequence on the free axis); the host passes x/a pre-transposed
and reassembles the output. Softmax runs on S^T tiles: the row-sum over keys
is a ones-matmul partition reduction accumulated in PSUM; no max subtraction
is needed because q/k rows are l2-normalized (|scores*scale| <= ~0.2 for
unit scales; fp32 exp is safe far beyond that).

Performance notes:
  - All matmul operands are bf16 (1 PE cycle/row vs fp32's 4); PSUM stays
    fp32. Inputs/weights/rotary tables are cast to bf16 on the host, which
    also halves the input DMA traffic.
  - The learned qk scales are folded into per-head rotary tables on the
    host (valid because l2norm's 1/||q|| is uniform across the head dim and
    commutes with the rotation), so the norm is a plain multiply by
    1/sqrt(sum q^2), computed with a single fused Abs_reciprocal_sqrt
    activation (Copy/Square/AbsRsqrt share one Act table; Exp is the only
    table switch, once, at attention start).
  - V is transposed with one bf16 XBAR DMA-transpose per half (3D output
    splits the key index into (jt, partition)); no PE/PSUM/DVE cost.
  - Both heads' rotary packs are single DMAs ([P, 2, n] head-minor tiles);
    Walrus requires identical start partitions for all DVE operands, so
    these half-swaps must go through SBUF->SBUF DMA.
  - Program order per batch: proj -> vtrans -> norm -> rotary, then both
    attentions. pj(4 banks) + nm(4) coexist in PSUM; attention then takes
    all 8 (512-wide score chunks, bufs=4, so exp never stalls the PE).
"""

import numpy as np
from contextlib import ExitStack

import concourse.bass as bass
from concourse import bacc
import concourse.mybir as mybir
import concourse.tile as tile

F32 = mybir.dt.float32
BF16 = mybir.dt.bfloat16
AF = mybir.ActivationFunctionType
ALU = mybir.AluOpType

P = 128
B, N, DIM = 2, 2048, 1024
HEADS, DH, ROT = 16, 64, 128
NCORES, HPC = 8, 2          # 2 heads per core
KT = DIM // P               # 8 contraction tiles
SM_SCALE = float(1.0 / np.sqrt(ROT))


def build_nc(n=N, nb=B):
    CH = min(512, n)        # matmul moving-operand chunk (1 PSUM bank fp32)
    NCH = n // CH
    SU = min(1024, n)       # attention superunit width (2 PSUM banks)
    NSU = n // SU
    SUC = SU // CH
    NJT = n // P            # key tiles

    nc = bacc.Bacc()
    dp = nc.declare_dram_parameter
    xT = dp("xT", [nb, DIM, n], BF16, isOutput=False)
    aT = dp("aT", [nb, DIM, n], BF16, isOutput=False)
    wqx = dp("wqx", [DIM, P], BF16, isOutput=False)
    wqa = dp("wqa", [DIM, P], BF16, isOutput=False)
    wkvx = dp("wkvx", [DIM, P], BF16, isOutput=False)   # cols [k_x | v_x]
    wkva = dp("wkva", [DIM, P], BF16, isOutput=False)   # cols [v_a | k_a] (host-permuted)
    cosq = dp("cosq", [P, HPC, n], BF16, isOutput=False)   # cos * q-scale, per head
    sinq = dp("sinq", [P, HPC, n], BF16, isOutput=False)   # signed sin * swapped q-scale
    cosk = dp("cosk", [P, n], BF16, isOutput=False)
    sink = dp("sink", [P, n], BF16, isOutput=False)
    out = dp("out", [nb, HPC, ROT, n], F32, isOutput=True)

    with ExitStack() as ctx:
        tc = ctx.enter_context(tile.TileContext(nc))
        consts = ctx.enter_context(tc.tile_pool(name="consts", bufs=1))
        sb = ctx.enter_context(tc.tile_pool(name="sb", bufs=1))

        ones = consts.tile([P, P], BF16)
        nc.vector.memset(ones, 1.0)
        eps_sb = consts.tile([P, 1], F32)
        nc.vector.memset(eps_sb, 1e-24)

        w_sb = {}
        for name, hdl in (("wqx", wqx), ("wkvx", wkvx), ("wqa", wqa), ("wkva", wkva)):
            t = consts.tile([P, KT, P], BF16, name=f"w_{name}")
            nc.gpsimd.dma_start(out=t, in_=hdl[:].rearrange("(kt p) m -> p kt m", p=P))
            w_sb[name] = t

        cosq_sb = consts.tile([P, HPC, n], BF16)
        nc.gpsimd.dma_start(out=cosq_sb, in_=cosq[:])
        sinq_sb = consts.tile([P, HPC, n], BF16)
        nc.gpsimd.dma_start(out=sinq_sb, in_=sinq[:])
        cosk_sb = consts.tile([P, n], BF16)
        nc.gpsimd.dma_start(out=cosk_sb, in_=cosk[:])
        sink_sb = consts.tile([P, n], BF16)
        nc.gpsimd.dma_start(out=sink_sb, in_=sink[:])

        # per-batch state
        QX, QA, KVX, KVA = {}, {}, {}, {}
        V = {}             # per b: [P, NJT, P] transposed values
        QH = {}            # per b: [P, HPC, n] packed+rotated q (both heads)
        KN = {}            # per b: packed+rotated k

        KTS = {}           # (b, src-name) -> list of input tiles

        def load(b):
            # issue all of batch b's input-tile DMAs; hoisted so the SP
            # HWDGE FIFO is never head-of-line blocked by later transfers
            # whose dependencies resolve late.
            for nmsrc, src in (("x", xT), ("a", aT)):
                kts = []
                for ki in range(KT):
                    t = sb.tile([P, n], BF16, tag="ktile", bufs=10)
                    nc.sync.dma_start(out=t, in_=src[b, ki * P:(ki + 1) * P, :])
                    kts.append(t)
                KTS[(b, nmsrc)] = kts

        def proj_pass(b, srcname, pj):
            kts = KTS[(b, srcname)]
            if srcname == "x":
                wq_t, wkv_t = w_sb["wqx"], w_sb["wkvx"]
                qdst = QX[b] = sb.tile([P, n], BF16, tag="proj", bufs=8,
                                       name=f"QX{b}")
                kvdst = KVX[b] = sb.tile([P, n], BF16, tag="proj", bufs=8,
                                         name=f"KVX{b}")
            else:
                wq_t, wkv_t = w_sb["wqa"], w_sb["wkva"]
                qdst = QA[b] = sb.tile([P, n], BF16, tag="proj", bufs=8,
                                       name=f"QA{b}")
                kvdst = KVA[b] = sb.tile([P, n], BF16, tag="proj", bufs=8,
                                         name=f"KVA{b}")
            # chunk pairs x ktile halves: the first matmuls only need the
            # first half of the input tiles, so compute starts earlier.
            CPS = 2 if NCH % 2 == 0 else 1
            for cp in range(NCH // CPS):
                ps = [(pj.tile([P, CH], F32, tag="pq", bufs=2, name=f"psq{ci}"),
                       pj.tile([P, CH], F32, tag="pk", bufs=2, name=f"psk{ci}"))
                      for ci in range(CPS)]
                for kg in range(2):
                    for ci in range(CPS):
                        c = CPS * cp + ci
                        cs = slice(c * CH, (c + 1) * CH)
                        psQ, psK = ps[ci]
                        for ki in range(kg * KT // 2, (kg + 1) * KT // 2):
                            nc.tensor.matmul(psQ, wq_t[:, ki, :], kts[ki][:, cs],
                                             start=(ki == 0), stop=(ki == KT - 1))
                            nc.tensor.matmul(psK, wkv_t[:, ki, :], kts[ki][:, cs],
                                             start=(ki == 0), stop=(ki == KT - 1))
                for ci in range(CPS):
                    c = CPS * cp + ci
                    cs = slice(c * CH, (c + 1) * CH)
                    psQ, psK = ps[ci]
                    # PSUM->SBUF eviction on the Activation engine, which is
                    # otherwise idle before attention; keeps DVE free.
                    nc.scalar.activation(qdst[:, cs], psQ, AF.Copy,
                                         bias=0.0, scale=1.0)
                    nc.scalar.activation(kvdst[:, cs], psK, AF.Copy,
                                         bias=0.0, scale=1.0)

        def vtrans(b):
            # V[key, d]: cols 0:64 = v_x (KVX rows 64:128), cols 64:128 = v_a
            # (KVA rows 0:64).  bf16 XBAR DMA transpose, one instruction per
            # half: out[p, jt, d] = in[d, jt*128 + p].
            va = sb.tile([P, NJT, P], BF16, tag="vsb", bufs=2, name=f"v{b}")
            nc.sync.dma_start_transpose(out=va[:, :, 0:DH], in_=KVX[b][DH:P, :])
            nc.sync.dma_start_transpose(out=va[:, :, DH:P], in_=KVA[b][0:DH, :])
            V[b] = va

        def norm_pass(b, srcname, nm):
            # multiply by 1/sqrt(sum q^2 + eps); the learned scales live in
            # the rotary tables.  Q tensors normalized in place, K halves
            # written into the packed KN tile.  Called per projection pass
            # so each stream's Act work starts as soon as its source tile
            # is evicted from PSUM.
            if (b, "kn") not in KTS:
                KTS[(b, "kn")] = KN[b] = sb.tile([P, n], BF16, tag="kn",
                                                 bufs=2, name=f"KN{b}")
            if srcname == "x":
                streams = ((QX[b], QX[b], (0, P)), (KVX[b], KN[b], (0, DH)))
            else:
                streams = ((QA[b], QA[b], (0, P)), (KVA[b], KN[b], (DH, P)))
            CHN = min(2 * CH, n)   # norm chunk: <=2 PSUM banks per psr buffer
            for src, dst, (r0, r1) in streams:
                q2 = sb.tile([P, n], BF16, tag="q2", bufs=2)
                rcp = sb.tile([P, n], BF16, tag="rcp", bufs=2)
                nc.vector.tensor_mul(q2[r0:r1, :], src[r0:r1, :], src[r0:r1, :])
                for c in range(n // CHN):
                    cs = slice(c * CHN, (c + 1) * CHN)
                    psr = nm.tile([P, CHN], F32, tag="r", bufs=2)
                    for h0 in range(r0, r1, DH):
                        h1 = h0 + DH
                        # matmul outputs must stay within one PSUM bank
                        for cc in range(CHN // CH):
                            el = slice(cc * CH, (cc + 1) * CH)
                            qs = slice(c * CHN + cc * CH, c * CHN + (cc + 1) * CH)
                            nc.tensor.matmul(psr[h0:h1, el], ones[h0:h1, 0:DH],
                                             q2[h0:h1, qs], start=True, stop=True)
                    # fused 1/sqrt on the Act engine (same table as Copy and
                    # Square -> no act-table reloads in the whole front)
                    nc.scalar.activation(rcp[r0:r1, cs], psr[r0:r1, :],
                                         AF.Abs_reciprocal_sqrt,
                                         bias=eps_sb[r0:r1, :], scale=1.0)
                nc.vector.tensor_mul(dst[r0:r1, :], src[r0:r1, :], rcp[r0:r1, :])

        def rotary(b):
            # rot(t)[0:64] = t[0:64]*cos64 - t[64:128]*sin64
            # rot(t)[64:128] = t[64:128]*cos64 + t[0:64]*sin64
            # The tables carry the sign and the learned scales. Walrus needs
            # identical start partitions for DVE operands, so the half-swapped
            # companions are built with SBUF->SBUF DMAs (one per source
            # covering both heads; dest is head-minor [P, HPC, n]).
            qh = sb.tile([P, HPC, n], BF16, tag="qh", bufs=2, name=f"qh{b}")
            qsw = sb.tile([P, HPC, n], BF16, tag="qsw", bufs=1, name=f"qsw{b}")
            # the SP HWDGE FIFO is drained of input tiles by the time these
            # are ready (loads are hoisted), so SP carries the packs.
            # (SBUF APs cannot move the partition axis into a free dim, so
            # this is one DMA per source half per head.)
            for h in range(HPC):
                hs = slice(h * DH, (h + 1) * DH)
                nc.sync.dma_start(out=qh[0:DH, h, :], in_=QX[b][hs, :])
                nc.sync.dma_start(out=qh[DH:P, h, :], in_=QA[b][hs, :])
                nc.sync.dma_start(out=qsw[0:DH, h, :], in_=QA[b][hs, :])
                nc.sync.dma_start(out=qsw[DH:P, h, :], in_=QX[b][hs, :])
            tc_ = sb.tile([P, HPC, n], BF16, tag="rt", bufs=1)
            nc.vector.tensor_mul(tc_, qh, cosq_sb)
            nc.vector.tensor_mul(qsw, qsw, sinq_sb)
            nc.vector.tensor_add(qh, tc_, qsw)
            QH[b] = qh

            ksw = sb.tile([P, n], BF16, tag="ksw", bufs=2, name=f"ksw{b}")
            nc.sync.dma_start(out=ksw[0:DH, :], in_=KN[b][DH:P, :])
            nc.sync.dma_start(out=ksw[DH:P, :], in_=KN[b][0:DH, :])
            tck = sb.tile([P, n], BF16, tag="rtk", bufs=1)
            nc.vector.tensor_mul(tck, KN[b], cosk_sb)
            nc.vector.tensor_mul(ksw, ksw, sink_sb)
            nc.vector.tensor_add(KN[b], tck, ksw)

        def attn(b, at):
            krot = KN[b]
            for h in range(HPC):
                qr = QH[b]
                for su in range(NSU):
                    ps_o = at.tile([P, SU], F32, tag="o", bufs=1)
                    # softmax denominator: accumulate sum_jt(es) with cheap
                    # bf16 DVE adds, then ONE ones-matmul partition-reduce on
                    # the pre-summed tile -- removes the per-tile rowsum
                    # matmul (1/3 of attention PE work).
                    acc = sb.tile([P, SU], BF16, tag="acc", bufs=2)
                    for jt in range(NJT):
                        js = slice(jt * P, (jt + 1) * P)
                        ps_s = at.tile([P, SU], F32, tag="s", bufs=3)
                        for cc in range(SUC):
                            el = slice(cc * CH, (cc + 1) * CH)
                            il = slice(su * SU + cc * CH, su * SU + (cc + 1) * CH)
                            nc.tensor.matmul(ps_s[:, el], krot[:, js],
                                             qr[:, h, il], start=True, stop=True)
                        es = sb.tile([P, SU], BF16, tag="es", bufs=3)
                        nc.scalar.activation(es, ps_s, AF.Exp, bias=0.0,
                                             scale=SM_SCALE)
                        if jt == 0:
                            nc.vector.tensor_copy(acc, es)
                        else:
                            nc.vector.tensor_add(acc, acc, es)
                        for cc in range(SUC):
                            el = slice(cc * CH, (cc + 1) * CH)
                            nc.tensor.matmul(ps_o[:, el], V[b][:, jt, :],
                                             es[:, el],
                                             start=(jt == 0), stop=(jt == NJT - 1))
                    psd = at.tile([P, SU], F32, tag="s", bufs=3)
                    for cc in range(SUC):
                        el = slice(cc * CH, (cc + 1) * CH)
                        nc.tensor.matmul(psd[:, el], ones, acc[:, el],
                                         start=True, stop=True)
                    rec = sb.tile([P, SU], F32, tag="rec", bufs=1)
                    nc.vector.reciprocal(rec, psd)
                    on = sb.tile([P, SU], F32, tag="on", bufs=2)
                    nc.vector.tensor_mul(on, ps_o, rec)
                    nc.sync.dma_start(out=out[b, h, :, su * SU:(su + 1) * SU],
                                      in_=on)

        # ---- program order: per-batch proj -> vtrans -> norm -> rotary
        # (rotary right after its norm keeps the DVE FIFO from head-of-line
        # blocking batch 0's rotary behind batch 1's norm), then attention
        # (needs all 8 PSUM banks -> pj/nm must close first).
        with tc.tile_pool(name="pj", bufs=1, space="PSUM") as pj, \
             tc.tile_pool(name="nm", bufs=1, space="PSUM") as nm:
            for b in range(nb):
                load(b)
            for b in range(nb):
                proj_pass(b, "x", pj)
                norm_pass(b, "x", nm)
                proj_pass(b, "a", pj)
                norm_pass(b, "a", nm)
                vtrans(b)
                rotary(b)
        with tc.tile_pool(name="at", bufs=1, space="PSUM") as at:
            for b in range(nb):
                attn(b, at)
    nc.finalize()
    return nc


# ---------------------------------------------------------------------------
# host side
# ---------------------------------------------------------------------------

_NC_CACHE = {}


def get_nc(n=N, nb=B):
    key = (n, nb)
    if key not in _NC_CACHE:
        _NC_CACHE[key] = build_nc(n, nb)
    return _NC_CACHE[key]


def rotary_tables(n):
    inv_freq = 1.0 / (10000.0 ** (np.arange(0, ROT, 2, dtype=np.float64) / ROT))
    freqs = np.outer(np.arange(n, dtype=np.float64), inv_freq)  # [n, 64]
    cos64 = np.cos(freqs).T.astype(np.float32)                  # [64, n]
    sin64 = np.sin(freqs).T.astype(np.float32)
    cos2 = np.concatenate([cos64, cos64], 0)                    # [128, n]
    sin2 = np.concatenate([-sin64, sin64], 0)                   # signed
    return cos2, sin2


def _bf16(a):
    import ml_dtypes
    return np.ascontiguousarray(np.asarray(a, dtype=np.float32)).astype(
        ml_dtypes.bfloat16)


def prep_in_maps(inputs, n=N, nb=B, ncores=NCORES):
    g = {k: np.ascontiguousarray(np.asarray(v, dtype=np.float32))
         for k, v in inputs.items()}
    xT = _bf16(g["x"].transpose(0, 2, 1))
    aT = _bf16(g["a"].transpose(0, 2, 1))
    wkvx = _bf16(g["Wkv_x"].T)                                     # cols [kx|vx]
    wkva = _bf16(
        np.concatenate([g["Wkv_a"][DH:2 * DH], g["Wkv_a"][0:DH]], 0).T)  # cols [va|ka]
    cos2, sin2 = rotary_tables(n)

    # fold the learned qk scales into the rotary tables:
    #   rot(qn * sc)[d] = cos2[d]*sc[d]*qn[d] + sin2[d]*sc[sigma(d)]*qn[sigma(d)]
    # where sigma swaps the two 64-halves; qn is the unscaled l2-normalized q.
    sck = np.concatenate([g["kx_scale"][0, 0], g["ka_scale"][0, 0]])       # [128]
    swk = np.concatenate([g["ka_scale"][0, 0], g["kx_scale"][0, 0]])
    cosk = cos2 * sck[:, None]
    sink = sin2 * swk[:, None]

    shared = dict(xT=xT, aT=aT, wkvx=wkvx, wkva=wkva,
                  cosk=_bf16(cosk), sink=_bf16(sink))
    in_maps = []
    for c in range(ncores):
        h0 = c * HPC
        m = dict(shared)
        m["wqx"] = _bf16(g["Wq_x"][h0 * DH:(h0 + HPC) * DH].T)
        m["wqa"] = _bf16(g["Wq_a"][h0 * DH:(h0 + HPC) * DH].T)
        cq = np.empty((P, HPC, n), np.float32)
        sq = np.empty((P, HPC, n), np.float32)
        for h in range(HPC):
            scq = np.concatenate([g["qx_scale"][h0 + h, 0], g["qa_scale"][h0 + h, 0]])
            swq = np.concatenate([g["qa_scale"][h0 + h, 0], g["qx_scale"][h0 + h, 0]])
            cq[:, h, :] = cos2 * scq[:, None]
            sq[:, h, :] = sin2 * swq[:, None]
        m["cosq"] = _bf16(cq)
        m["sinq"] = _bf16(sq)
        in_maps.append(m)
    return in_maps


def gather_out(results, n=N, nb=B, ncores=NCORES):
    full = np.empty((nb, n, HEADS * ROT), np.float32)
    for c in range(ncores):
        o = np.asarray(results[c]["out"])          # [nb, HPC, ROT, n]
        for h in range(HPC):
            gh = c * HPC + h
            full[:, :, gh * ROT:(gh + 1) * ROT] = o[:, h].transpose(0, 2, 1)
    return full


def kernel(**inputs):
    from concourse.bass_utils import run_bass_kernel_spmd
    nc = get_nc(N, B)
    in_maps = prep_in_maps(inputs, N, B, NCORES)
    res = run_bass_kernel_spmd(nc, in_maps, list(range(NCORES)))
    return gather_out(res.results, N, B, NCORES)


if __name__ == "__main__":
    nc = build_nc(256, 1)
    print("build ok")
